# revision 12
# baseline (speedup 1.0000x reference)
"""Trainium2 Bass kernel for pairwise DiceLoss (v4).

Math (per reference):
    an[b,k,:]  = am[b,k,:] / (S[b,k] + EPS),  S = row sums of am
    gram_n     = an . an^T per batch          (16 x 16 per batch)
    dice[b,k,l]= (2*gram_n + 0.1) / (a[b,k] + a[b,l] + 0.1),  a = S/(S+EPS)
    loss       = mean over masked (k<l, same batch) pairs and batches

fp32-exact algebra: S ~ 32768 so S + 1e-8 == S in fp32 => a == 1.0 exactly
and the dice denominator is the constant 2.1 (identical to the reference's
own fp32 arithmetic to ~1e-7).  The device returns only the per-column sums
of mask*G[m,j]*r_m*r_j; host applies the affine map to the loss.

Device strategy (per core, 8 batches x 16 slots = 128 rows = 128 SBUF
partitions; every choice below is trace-measurement-driven, see v1-v3):
  - One full 128-row Gram via 512 accumulating PE matmuls (K=128 pixels per
    chunk).  The PE issue floor is ~34ns per LDWEIGHTS+MATMUL pair, so
    tile_position splits lose (v2 measured 2x worse); the rhs stream
    (1 col/cycle @2.4GHz warm) is the binding resource at ~53ns/chunk.
  - fp8e4m3 input (4x less HBM traffic; error cancels over 65536-element
    contractions).
  - Rows reordered so the 8 slot-0 rows come first: a column j is needed
    only for pairs m<j in the same batch, so slot-0 columns produce nothing
    -> rhs streams only columns 8..128 (120 data + ones), N=121 not 129.
  - Warmup: junk matmuls with no DMA dependency issue from program start,
    carrying the PE through the HAM activity window so real matmuls run at
    2.4GHz (v3 measured warm flip at 11.1us vs 14.4 without).
  - DMA: early tiles small (PE starts ~9.2us; early DMA only sustains
    ~0.3MB/us so large early tiles starve the PE - v3 measured 4us of
    boundary stalls), growing once the pipe is saturated.
  - Epilogue: r = 1/S (EPS is below fp32 ulp of S); the pair mask is folded
    into the broadcast matmul weights (maskT*r on DVE, then one bf16 matmul
    against a permuted identity); final partition-reduce via a ones-column
    matmul so the output is a single-partition [1,121] row -> one DMA
    descriptor (v3's [128,1] output = 128 4-byte descriptors cost ~6.5us in
    HBM write receipts).
Host: loss = (2*sum(out) + 0.1*npairs_total) / 2.1 / npairs_total.
"""

import os

import numpy as np

B, K, N = 64, 16, 65536
NCORES = 8
BPC = B // NCORES  # 8 batches per core
R = BPC * K  # 128 data rows per core
P = 128  # SBUF partitions
C_PER_P = N // P  # 512 pixel-chunks of 128
NC = R - BPC + 1  # 121 streamed columns: 120 slot>0 rows + ones
TILES = [16, 48, 64, 96, 128, 160]  # sums to C_PER_P
WARMUP = int(os.environ.get("KERNEL_WARMUP", "17"))
SMOOTH = 0.1

_CACHE: dict = {}

# test.py reads this after calling kernel() to print HW exec time
LAST_RESULTS = None


def _row_order() -> np.ndarray:
    """Row permutation: the 8 slot-0 rows first, then slot 1..15 by batch."""
    first = [b * K for b in range(BPC)]
    rest = [b * K + k for b in range(BPC) for k in range(1, K)]
    return np.array(first + rest)


def _build_nc():
    import concourse.bacc as bacc
    import concourse.mybir as mybir
    import concourse.tile as tile

    f32 = mybir.dt.float32
    bf16 = mybir.dt.bfloat16
    xdt = mybir.dt.float8e4
    nc = bacc.Bacc("TRN2", target_bir_lowering=False)

    x = nc.dram_tensor("x", [P, C_PER_P, R + 1], xdt, kind="ExternalInput")
    consts = nc.dram_tensor("consts", [P, P + NC + 1], bf16, kind="ExternalInput")
    out_d = nc.dram_tensor("out", [1, NC], f32, kind="ExternalOutput")

    with tile.TileContext(nc) as tc:
        with (
            tc.tile_pool(name="xp", bufs=1) as xp,
            tc.tile_pool(name="sg", bufs=1) as sg,
            tc.tile_pool(name="ps", bufs=1, space="PSUM") as ps,
            tc.tile_pool(name="ps2", bufs=1, space="PSUM") as ps2,
            tc.tile_pool(name="wps", bufs=1, space="PSUM") as wps,
        ):
            # ---- PE warmup: junk matmuls with no DMA dependency ----
            wjunk = sg.tile([P, 16], xdt, name="wjunk")
            nc.gpsimd.memset(wjunk[:], 0.5)
            w_ps = wps.tile([P, 128], f32)
            for _ in range(WARMUP):
                nc.tensor.matmul(
                    w_ps[0:16, :],
                    wjunk[:, 0:16],
                    wjunk[:, 0:1].to_broadcast([P, 128]),
                    start=True,
                    stop=True,
                )

            # ---- input tiles (all resident; 66KB/partition at fp8) ----
            # alternate the two HWDGE queues (sync, scalar) so descriptor
            # generation for consecutive tiles overlaps
            xts = []
            off = 0
            for t, cc in enumerate(TILES):
                xt = xp.tile([P, cc, R + 1], xdt, name=f"xt{t}")
                eng = nc.sync if t % 2 == 0 else nc.scalar
                eng.dma_start(xt[:], x[:, off : off + cc, :])
                xts.append((xt, off, cc))
                off += cc
            # epilogue-only data, off the critical path
            consts_sb = sg.tile([P, P + NC + 1], bf16)
            nc.scalar.dma_start(consts_sb[:], consts[:, :])
            maskTc = consts_sb[:, 0:P]  # maskT[q, m] = mask[m, col(q)]
            identc = consts_sb[:, P : P + NC]  # ident[q, j] = (q == 8+j)
            onesc = consts_sb[:, P + NC : P + NC + 1]

            # ---- Gram accumulation: 512 x (lhsT [128,128], rhs [128,121]) --
            g_ps = ps.tile([P, NC], f32)
            for xt, off, cc in xts:
                for c in range(cc):
                    nc.tensor.matmul(
                        g_ps[:],
                        xt[:, c, 0:R],
                        xt[:, c, BPC : R + 1],
                        start=(off + c == 0),
                        stop=(off + c == C_PER_P - 1),
                    )

            # ---- epilogue ----
            # bf16 is plenty: the gram term is ~0.1% of the loss and the
            # tolerance is 2e-2; measured end-to-end error stays ~1e-6
            s_ps = g_ps[:, NC - 1 : NC]  # S[row] in PSUM (ones column)
            pack = sg.tile([P, 1], f32)
            nc.vector.reciprocal(pack[:], s_ps)  # r = 1/S
            rcol = pack[:, 0:1]
            with nc.allow_low_precision(reason="dice gram term, tol 2e-2"):
                # rBm[m, j] = mask[m, j] * r[col(j)] via one bf16 matmul
                # whose weights are maskT pre-scaled by r on DVE
                rmulb = sg.tile([P, P], bf16)
                nc.vector.tensor_scalar_mul(rmulb[:], maskTc, rcol)
                rBm = ps2.tile([P, NC], f32)
                nc.tensor.matmul(
                    rBm[:], rmulb[:], identc, start=True, stop=True
                )

                t1 = sg.tile([P, NC], bf16)
                nc.vector.tensor_scalar_mul(t1[:], g_ps[:], rcol)  # G*r_m
                nc.vector.tensor_mul(t1[:], t1[:], rBm[:])  # *r_j*mask

                # partition-reduce to one row: the output DMA is 1 descriptor
                nc.tensor.matmul(
                    w_ps[0:1, 0:NC], onesc, t1[:], start=True, stop=True
                )
            osb = sg.tile([P, NC], f32, name="osb")
            nc.vector.tensor_copy(out=osb[0:1, :], in_=w_ps[0:1, 0:NC])
            nc.sync.dma_start(out_d[:, :], osb[0:1, :])

    nc.compile()
    return nc


def _make_consts() -> np.ndarray:
    """[P, P+NC+1] bf16: maskT | permuted identity | ones column."""
    import ml_dtypes

    order = _row_order()
    m_row = order  # [128] original row id per out partition
    j_row = order[BPC:]  # [120] original row id per streamed data column
    mb, mk = m_row // K, m_row % K
    jb, jk = j_row // K, j_row % K
    mask = (mb[:, None] == jb[None, :]) & (mk[:, None] < jk[None, :])
    consts = np.zeros((P, P + NC + 1), dtype=ml_dtypes.bfloat16)
    # maskT[q, m] = mask[m, q-8] for q in 8..127
    consts[BPC:, 0:P] = mask.T.astype(ml_dtypes.bfloat16)
    for j in range(NC - 1):
        consts[BPC + j, P + j] = 1.0
    consts[:, P + NC] = 1.0
    return consts


def _shard_core(am_rows: np.ndarray) -> np.ndarray:
    """[8, 16, 65536] f32 -> [P, C_PER_P, R+1] fp8 device layout."""
    import ml_dtypes

    ndt = ml_dtypes.float8_e4m3
    xr = np.empty((R + 1, N), dtype=ndt)
    xr[0:R] = am_rows.reshape(R, N)[_row_order()].astype(ndt)
    xr[R] = 1.0
    # pixel n = p*C_PER_P + c ; [bk, p, c] -> [p, c, bk]
    xt = xr.reshape(R + 1, P, C_PER_P).transpose(1, 2, 0)
    return np.ascontiguousarray(xt)


def kernel(am: np.ndarray) -> np.ndarray:
    global LAST_RESULTS
    from concourse.bass_utils import run_bass_kernel_spmd

    if "nc" not in _CACHE:
        _CACHE["nc"] = _build_nc()
        _CACHE["consts"] = _make_consts()
    nc = _CACHE["nc"]
    consts = _CACHE["consts"]

    am = np.ascontiguousarray(np.asarray(am), dtype=np.float32)
    assert am.shape == (B, K, N)

    in_maps = []
    for core in range(NCORES):
        rows = am[core * BPC : (core + 1) * BPC]
        in_maps.append({"x": _shard_core(rows), "consts": consts})

    trace = bool(int(os.environ.get("KERNEL_TRACE", "0")))
    res = run_bass_kernel_spmd(
        nc, in_maps, core_ids=list(range(NCORES)), trace=trace
    )
    LAST_RESULTS = res

    tsum = float(
        np.sum(
            np.array([r["out"][0, :] for r in res.results], dtype=np.float64)
        )
    )
    npairs = K * (K - 1) // 2
    ntot = B * npairs  # 7680 masked pairs overall
    loss = (2.0 * tsum + SMOOTH * ntot) / (2.0 + SMOOTH) / ntot
    return np.float32(loss)


# revision 16
# speedup vs baseline: 1.0041x; 1.0041x over previous
"""Trainium2 Bass kernel for pairwise DiceLoss (v4).

Math (per reference):
    an[b,k,:]  = am[b,k,:] / (S[b,k] + EPS),  S = row sums of am
    gram_n     = an . an^T per batch          (16 x 16 per batch)
    dice[b,k,l]= (2*gram_n + 0.1) / (a[b,k] + a[b,l] + 0.1),  a = S/(S+EPS)
    loss       = mean over masked (k<l, same batch) pairs and batches

fp32-exact algebra: S ~ 32768 so S + 1e-8 == S in fp32 => a == 1.0 exactly
and the dice denominator is the constant 2.1 (identical to the reference's
own fp32 arithmetic to ~1e-7).  The device returns only the per-column sums
of mask*G[m,j]*r_m*r_j; host applies the affine map to the loss.

Device strategy (per core, 8 batches x 16 slots = 128 rows = 128 SBUF
partitions; every choice below is trace-measurement-driven, see v1-v3):
  - One full 128-row Gram via 512 accumulating PE matmuls (K=128 pixels per
    chunk).  The PE issue floor is ~34ns per LDWEIGHTS+MATMUL pair, so
    tile_position splits lose (v2 measured 2x worse); the rhs stream
    (1 col/cycle @2.4GHz warm) is the binding resource at ~53ns/chunk.
  - fp8e4m3 input (4x less HBM traffic; error cancels over 65536-element
    contractions).
  - Rows reordered so the 8 slot-0 rows come first: a column j is needed
    only for pairs m<j in the same batch, so slot-0 columns produce nothing
    -> rhs streams only columns 8..128 (120 data + ones), N=121 not 129.
  - Warmup: junk matmuls with no DMA dependency issue from program start,
    carrying the PE through the HAM activity window so real matmuls run at
    2.4GHz (v3 measured warm flip at 11.1us vs 14.4 without).
  - DMA: early tiles small (PE starts ~9.2us; early DMA only sustains
    ~0.3MB/us so large early tiles starve the PE - v3 measured 4us of
    boundary stalls), growing once the pipe is saturated.
  - Epilogue: r = 1/S (EPS is below fp32 ulp of S); the pair mask is folded
    into the broadcast matmul weights (maskT*r on DVE, then one bf16 matmul
    against a permuted identity); final partition-reduce via a ones-column
    matmul so the output is a single-partition [1,121] row -> one DMA
    descriptor (v3's [128,1] output = 128 4-byte descriptors cost ~6.5us in
    HBM write receipts).
Host: loss = (2*sum(out) + 0.1*npairs_total) / 2.1 / npairs_total.
"""

import os

import numpy as np

B, K, N = 64, 16, 65536
NCORES = 8
BPC = B // NCORES  # 8 batches per core
R = BPC * K  # 128 data rows per core
P = 128  # SBUF partitions
C_PER_P = N // P  # 512 pixel-chunks of 128
NC = R - BPC + 1  # 121 streamed columns: 120 slot>0 rows + ones
TILES = [16, 32, 48, 64, 80, 96, 88, 88]  # sums to C_PER_P
WARMUP = int(os.environ.get("KERNEL_WARMUP", "24"))
SMOOTH = 0.1

_CACHE: dict = {}

# test.py reads this after calling kernel() to print HW exec time
LAST_RESULTS = None


def _row_order() -> np.ndarray:
    """Row permutation: the 8 slot-0 rows first, then slot 1..15 by batch."""
    first = [b * K for b in range(BPC)]
    rest = [b * K + k for b in range(BPC) for k in range(1, K)]
    return np.array(first + rest)


def _build_nc():
    import concourse.bacc as bacc
    import concourse.mybir as mybir
    import concourse.tile as tile

    f32 = mybir.dt.float32
    bf16 = mybir.dt.bfloat16
    xdt = mybir.dt.float8e4
    nc = bacc.Bacc("TRN2", target_bir_lowering=False)

    x = nc.dram_tensor("x", [P, C_PER_P, R + 1], xdt, kind="ExternalInput")
    consts = nc.dram_tensor("consts", [P, P + NC + 1], bf16, kind="ExternalInput")
    out_d = nc.dram_tensor("out", [1, NC], f32, kind="ExternalOutput")

    with tile.TileContext(nc) as tc:
        with (
            tc.tile_pool(name="xp", bufs=1) as xp,
            tc.tile_pool(name="sg", bufs=1) as sg,
            tc.tile_pool(name="ps", bufs=1, space="PSUM") as ps,
            tc.tile_pool(name="ps2", bufs=1, space="PSUM") as ps2,
            tc.tile_pool(name="wps", bufs=1, space="PSUM") as wps,
        ):
            # ---- PE warmup: junk matmuls with no DMA dependency ----
            wjunk = sg.tile([P, 16], xdt, name="wjunk")
            nc.gpsimd.memset(wjunk[:], 0.5)
            w_ps = wps.tile([P, 128], f32)
            for _ in range(WARMUP):
                nc.tensor.matmul(
                    w_ps[0:16, :],
                    wjunk[:, 0:16],
                    wjunk[:, 0:1].to_broadcast([P, 128]),
                    start=True,
                    stop=True,
                )

            # ---- input tiles (all resident; 66KB/partition at fp8) ----
            # each tile is split by partition halves across the two HWDGE
            # queues (sync, scalar): double descriptor throughput while
            # keeping arrival order = consumption order (v5 measured that
            # alternating whole tiles across queues reorders arrivals and
            # stalls the PE into HAM re-throttle)
            xts = []
            off = 0
            for t, cc in enumerate(TILES):
                xt = xp.tile([P, cc, R + 1], xdt, name=f"xt{t}")
                half = P // 2
                nc.sync.dma_start(
                    xt[0:half, :, :], x[0:half, off : off + cc, :]
                )
                nc.scalar.dma_start(
                    xt[half:P, :, :], x[half:P, off : off + cc, :]
                )
                xts.append((xt, off, cc))
                off += cc
            # epilogue-only data, off the critical path
            consts_sb = sg.tile([P, P + NC + 1], bf16)
            nc.scalar.dma_start(consts_sb[:], consts[:, :])
            maskTc = consts_sb[:, 0:P]  # maskT[q, m] = mask[m, col(q)]
            identc = consts_sb[:, P : P + NC]  # ident[q, j] = (q == 8+j)
            onesc = consts_sb[:, P + NC : P + NC + 1]

            # ---- Gram accumulation: 512 x (lhsT [128,128], rhs [128,121]) --
            g_ps = ps.tile([P, NC], f32)
            for xt, off, cc in xts:
                for c in range(cc):
                    nc.tensor.matmul(
                        g_ps[:],
                        xt[:, c, 0:R],
                        xt[:, c, BPC : R + 1],
                        start=(off + c == 0),
                        stop=(off + c == C_PER_P - 1),
                    )

            # ---- epilogue ----
            # bf16 is plenty: the gram term is ~0.1% of the loss and the
            # tolerance is 2e-2; measured end-to-end error stays ~1e-6
            s_ps = g_ps[:, NC - 1 : NC]  # S[row] in PSUM (ones column)
            pack = sg.tile([P, 1], f32)
            nc.vector.reciprocal(pack[:], s_ps)  # r = 1/S
            rcol = pack[:, 0:1]
            with nc.allow_low_precision(reason="dice gram term, tol 2e-2"):
                # rBm[m, j] = mask[m, j] * r[col(j)] via one bf16 matmul
                # whose weights are maskT pre-scaled by r on DVE
                rmulb = sg.tile([P, P], bf16)
                nc.vector.tensor_scalar_mul(rmulb[:], maskTc, rcol)
                rBm = ps2.tile([P, NC], f32)
                nc.tensor.matmul(
                    rBm[:], rmulb[:], identc, start=True, stop=True
                )

                t1 = sg.tile([P, NC], bf16)
                nc.vector.tensor_scalar_mul(t1[:], g_ps[:], rcol)  # G*r_m
                nc.vector.tensor_mul(t1[:], t1[:], rBm[:])  # *r_j*mask

                # partition-reduce to one row: the output DMA is 1 descriptor
                nc.tensor.matmul(
                    w_ps[0:1, 0:NC], onesc, t1[:], start=True, stop=True
                )
            osb = sg.tile([P, NC], f32, name="osb")
            nc.vector.tensor_copy(out=osb[0:1, :], in_=w_ps[0:1, 0:NC])
            nc.sync.dma_start(out_d[:, :], osb[0:1, :])

    nc.compile()
    return nc


def _make_consts() -> np.ndarray:
    """[P, P+NC+1] bf16: maskT | permuted identity | ones column."""
    import ml_dtypes

    order = _row_order()
    m_row = order  # [128] original row id per out partition
    j_row = order[BPC:]  # [120] original row id per streamed data column
    mb, mk = m_row // K, m_row % K
    jb, jk = j_row // K, j_row % K
    mask = (mb[:, None] == jb[None, :]) & (mk[:, None] < jk[None, :])
    consts = np.zeros((P, P + NC + 1), dtype=ml_dtypes.bfloat16)
    # maskT[q, m] = mask[m, q-8] for q in 8..127
    consts[BPC:, 0:P] = mask.T.astype(ml_dtypes.bfloat16)
    for j in range(NC - 1):
        consts[BPC + j, P + j] = 1.0
    consts[:, P + NC] = 1.0
    return consts


def _shard_core(am_rows: np.ndarray) -> np.ndarray:
    """[8, 16, 65536] f32 -> [P, C_PER_P, R+1] fp8 device layout."""
    import ml_dtypes

    ndt = ml_dtypes.float8_e4m3
    xr = np.empty((R + 1, N), dtype=ndt)
    xr[0:R] = am_rows.reshape(R, N)[_row_order()].astype(ndt)
    xr[R] = 1.0
    # pixel n = p*C_PER_P + c ; [bk, p, c] -> [p, c, bk]
    xt = xr.reshape(R + 1, P, C_PER_P).transpose(1, 2, 0)
    return np.ascontiguousarray(xt)


def kernel(am: np.ndarray) -> np.ndarray:
    global LAST_RESULTS
    from concourse.bass_utils import run_bass_kernel_spmd

    if "nc" not in _CACHE:
        _CACHE["nc"] = _build_nc()
        _CACHE["consts"] = _make_consts()
    nc = _CACHE["nc"]
    consts = _CACHE["consts"]

    am = np.ascontiguousarray(np.asarray(am), dtype=np.float32)
    assert am.shape == (B, K, N)

    in_maps = []
    for core in range(NCORES):
        rows = am[core * BPC : (core + 1) * BPC]
        in_maps.append({"x": _shard_core(rows), "consts": consts})

    trace = bool(int(os.environ.get("KERNEL_TRACE", "0")))
    res = run_bass_kernel_spmd(
        nc, in_maps, core_ids=list(range(NCORES)), trace=trace
    )
    LAST_RESULTS = res

    tsum = float(
        np.sum(
            np.array([r["out"][0, :] for r in res.results], dtype=np.float64)
        )
    )
    npairs = K * (K - 1) // 2
    ntot = B * npairs  # 7680 masked pairs overall
    loss = (2.0 * tsum + SMOOTH * ntot) / (2.0 + SMOOTH) / ntot
    return np.float32(loss)


# revision 18
# speedup vs baseline: 1.1899x; 1.1850x over previous
"""Trainium2 Bass kernel for pairwise DiceLoss (v4).

Math (per reference):
    an[b,k,:]  = am[b,k,:] / (S[b,k] + EPS),  S = row sums of am
    gram_n     = an . an^T per batch          (16 x 16 per batch)
    dice[b,k,l]= (2*gram_n + 0.1) / (a[b,k] + a[b,l] + 0.1),  a = S/(S+EPS)
    loss       = mean over masked (k<l, same batch) pairs and batches

fp32-exact algebra: S ~ 32768 so S + 1e-8 == S in fp32 => a == 1.0 exactly
and the dice denominator is the constant 2.1 (identical to the reference's
own fp32 arithmetic to ~1e-7).  The device returns only the per-column sums
of mask*G[m,j]*r_m*r_j; host applies the affine map to the loss.

Device strategy (per core, 8 batches x 16 slots = 128 rows = 128 SBUF
partitions; every choice below is trace-measurement-driven, see v1-v3):
  - One full 128-row Gram via 512 accumulating PE matmuls (K=128 pixels per
    chunk).  The PE issue floor is ~34ns per LDWEIGHTS+MATMUL pair, so
    tile_position splits lose (v2 measured 2x worse); the rhs stream
    (1 col/cycle @2.4GHz warm) is the binding resource at ~53ns/chunk.
  - fp8e4m3 input (4x less HBM traffic; error cancels over 65536-element
    contractions).
  - Rows reordered so the 8 slot-0 rows come first: a column j is needed
    only for pairs m<j in the same batch, so slot-0 columns produce nothing
    -> rhs streams only columns 8..128 (120 data + ones), N=121 not 129.
  - Warmup: junk matmuls with no DMA dependency issue from program start,
    carrying the PE through the HAM activity window so real matmuls run at
    2.4GHz (v3 measured warm flip at 11.1us vs 14.4 without).
  - DMA: early tiles small (PE starts ~9.2us; early DMA only sustains
    ~0.3MB/us so large early tiles starve the PE - v3 measured 4us of
    boundary stalls), growing once the pipe is saturated.
  - Epilogue: r = 1/S (EPS is below fp32 ulp of S); the pair mask is folded
    into the broadcast matmul weights (maskT*r on DVE, then one bf16 matmul
    against a permuted identity); final partition-reduce via a ones-column
    matmul so the output is a single-partition [1,121] row -> one DMA
    descriptor (v3's [128,1] output = 128 4-byte descriptors cost ~6.5us in
    HBM write receipts).
Host: loss = (2*sum(out) + 0.1*npairs_total) / 2.1 / npairs_total.
"""

import os

import numpy as np

B, K, N = 64, 16, 65536
NCORES = 8
BPC = B // NCORES  # 8 batches per core
R = BPC * K  # 128 data rows per core
P = 128  # SBUF partitions
C_PER_P = N // P  # 512 pixel-chunks of 128
NC = R - BPC + 1  # 121 streamed columns: 120 slot>0 rows + ones
# column-halves of each supertile go to the two HWDGE queues concurrently
TILES = [16, 32, 48, 64, 80, 96, 112, 64]  # sums to C_PER_P
WARMUP = int(os.environ.get("KERNEL_WARMUP", "24"))
SMOOTH = 0.1

_CACHE: dict = {}

# test.py reads this after calling kernel() to print HW exec time
LAST_RESULTS = None


def _row_order() -> np.ndarray:
    """Row permutation: the 8 slot-0 rows first, then slot 1..15 by batch."""
    first = [b * K for b in range(BPC)]
    rest = [b * K + k for b in range(BPC) for k in range(1, K)]
    return np.array(first + rest)


def _build_nc():
    import concourse.bacc as bacc
    import concourse.mybir as mybir
    import concourse.tile as tile

    f32 = mybir.dt.float32
    bf16 = mybir.dt.bfloat16
    xdt = mybir.dt.float8e4
    nc = bacc.Bacc("TRN2", target_bir_lowering=False)

    x = nc.dram_tensor("x", [P, C_PER_P, R + 1], xdt, kind="ExternalInput")
    consts = nc.dram_tensor("consts", [P, P + NC + 1], bf16, kind="ExternalInput")
    out_d = nc.dram_tensor("out", [1, NC], f32, kind="ExternalOutput")

    with tile.TileContext(nc) as tc:
        with (
            tc.tile_pool(name="xp", bufs=1) as xp,
            tc.tile_pool(name="sg", bufs=1) as sg,
            tc.tile_pool(name="ps", bufs=1, space="PSUM") as ps,
            tc.tile_pool(name="ps2", bufs=1, space="PSUM") as ps2,
            tc.tile_pool(name="wps", bufs=1, space="PSUM") as wps,
        ):
            # ---- PE warmup: junk matmuls with no DMA dependency ----
            wjunk = sg.tile([P, 16], xdt, name="wjunk")
            nc.gpsimd.memset(wjunk[:], 0.5)
            w_ps = wps.tile([P, 128], f32)
            for _ in range(WARMUP):
                nc.tensor.matmul(
                    w_ps[0:16, :],
                    wjunk[:, 0:16],
                    wjunk[:, 0:1].to_broadcast([P, 128]),
                    start=True,
                    stop=True,
                )

            # ---- input tiles (all resident; 66KB/partition at fp8) ----
            # each supertile's columns are split across the two HWDGE
            # queues (sync first half, scalar second half): two descriptor
            # generators run concurrently (one queue ramps at only
    # ~0.2MB/us on small descriptors) while arrival order stays equal to
            # the PE's consumption order (v5/v6 measured that whole-tile
            # alternation reorders arrivals and partition-splitting halves
            # throughput; both stall the PE into HAM re-throttle)
            xts = []
            off = 0
            for t, cc in enumerate(TILES):
                xt = xp.tile([P, cc, R + 1], xdt, name=f"xt{t}")
                half = cc // 2
                nc.sync.dma_start(
                    xt[:, 0:half, :], x[:, off : off + half, :]
                )
                nc.scalar.dma_start(
                    xt[:, half:cc, :], x[:, off + half : off + cc, :]
                )
                xts.append((xt, off, cc))
                off += cc
            # epilogue-only data, off the critical path
            consts_sb = sg.tile([P, P + NC + 1], bf16)
            nc.scalar.dma_start(consts_sb[:], consts[:, :])
            maskTc = consts_sb[:, 0:P]  # maskT[q, m] = mask[m, col(q)]
            identc = consts_sb[:, P : P + NC]  # ident[q, j] = (q == 8+j)
            onesc = consts_sb[:, P + NC : P + NC + 1]

            # ---- Gram accumulation: 512 x (lhsT [128,128], rhs [128,121]) --
            g_ps = ps.tile([P, NC], f32)
            for xt, off, cc in xts:
                for c in range(cc):
                    nc.tensor.matmul(
                        g_ps[:],
                        xt[:, c, 0:R],
                        xt[:, c, BPC : R + 1],
                        start=(off + c == 0),
                        stop=(off + c == C_PER_P - 1),
                    )

            # ---- epilogue ----
            # bf16 is plenty: the gram term is ~0.1% of the loss and the
            # tolerance is 2e-2; measured end-to-end error stays ~1e-6
            s_ps = g_ps[:, NC - 1 : NC]  # S[row] in PSUM (ones column)
            pack = sg.tile([P, 1], f32)
            nc.vector.reciprocal(pack[:], s_ps)  # r = 1/S
            rcol = pack[:, 0:1]
            with nc.allow_low_precision(reason="dice gram term, tol 2e-2"):
                # rBm[m, j] = mask[m, j] * r[col(j)] via one bf16 matmul
                # whose weights are maskT pre-scaled by r on DVE
                rmulb = sg.tile([P, P], bf16)
                nc.vector.tensor_scalar_mul(rmulb[:], maskTc, rcol)
                rBm = ps2.tile([P, NC], f32)
                nc.tensor.matmul(
                    rBm[:], rmulb[:], identc, start=True, stop=True
                )

                t1 = sg.tile([P, NC], bf16)
                nc.vector.tensor_scalar_mul(t1[:], g_ps[:], rcol)  # G*r_m
                nc.vector.tensor_mul(t1[:], t1[:], rBm[:])  # *r_j*mask

                # partition-reduce to one row: the output DMA is 1 descriptor
                nc.tensor.matmul(
                    w_ps[0:1, 0:NC], onesc, t1[:], start=True, stop=True
                )
            osb = sg.tile([P, NC], f32, name="osb")
            nc.vector.tensor_copy(out=osb[0:1, :], in_=w_ps[0:1, 0:NC])
            nc.sync.dma_start(out_d[:, :], osb[0:1, :])

    nc.compile()
    return nc


def _make_consts() -> np.ndarray:
    """[P, P+NC+1] bf16: maskT | permuted identity | ones column."""
    import ml_dtypes

    order = _row_order()
    m_row = order  # [128] original row id per out partition
    j_row = order[BPC:]  # [120] original row id per streamed data column
    mb, mk = m_row // K, m_row % K
    jb, jk = j_row // K, j_row % K
    mask = (mb[:, None] == jb[None, :]) & (mk[:, None] < jk[None, :])
    consts = np.zeros((P, P + NC + 1), dtype=ml_dtypes.bfloat16)
    # maskT[q, m] = mask[m, q-8] for q in 8..127
    consts[BPC:, 0:P] = mask.T.astype(ml_dtypes.bfloat16)
    for j in range(NC - 1):
        consts[BPC + j, P + j] = 1.0
    consts[:, P + NC] = 1.0
    return consts


def _shard_core(am_rows: np.ndarray) -> np.ndarray:
    """[8, 16, 65536] f32 -> [P, C_PER_P, R+1] fp8 device layout."""
    import ml_dtypes

    ndt = ml_dtypes.float8_e4m3
    xr = np.empty((R + 1, N), dtype=ndt)
    xr[0:R] = am_rows.reshape(R, N)[_row_order()].astype(ndt)
    xr[R] = 1.0
    # pixel n = p*C_PER_P + c ; [bk, p, c] -> [p, c, bk]
    xt = xr.reshape(R + 1, P, C_PER_P).transpose(1, 2, 0)
    return np.ascontiguousarray(xt)


def kernel(am: np.ndarray) -> np.ndarray:
    global LAST_RESULTS
    from concourse.bass_utils import run_bass_kernel_spmd

    if "nc" not in _CACHE:
        _CACHE["nc"] = _build_nc()
        _CACHE["consts"] = _make_consts()
    nc = _CACHE["nc"]
    consts = _CACHE["consts"]

    am = np.ascontiguousarray(np.asarray(am), dtype=np.float32)
    assert am.shape == (B, K, N)

    in_maps = []
    for core in range(NCORES):
        rows = am[core * BPC : (core + 1) * BPC]
        in_maps.append({"x": _shard_core(rows), "consts": consts})

    trace = bool(int(os.environ.get("KERNEL_TRACE", "0")))
    res = run_bass_kernel_spmd(
        nc, in_maps, core_ids=list(range(NCORES)), trace=trace
    )
    LAST_RESULTS = res

    tsum = float(
        np.sum(
            np.array([r["out"][0, :] for r in res.results], dtype=np.float64)
        )
    )
    npairs = K * (K - 1) // 2
    ntot = B * npairs  # 7680 masked pairs overall
    loss = (2.0 * tsum + SMOOTH * ntot) / (2.0 + SMOOTH) / ntot
    return np.float32(loss)


# revision 19
# speedup vs baseline: 1.2420x; 1.0438x over previous
"""Trainium2 Bass kernel for pairwise DiceLoss (v4).

Math (per reference):
    an[b,k,:]  = am[b,k,:] / (S[b,k] + EPS),  S = row sums of am
    gram_n     = an . an^T per batch          (16 x 16 per batch)
    dice[b,k,l]= (2*gram_n + 0.1) / (a[b,k] + a[b,l] + 0.1),  a = S/(S+EPS)
    loss       = mean over masked (k<l, same batch) pairs and batches

fp32-exact algebra: S ~ 32768 so S + 1e-8 == S in fp32 => a == 1.0 exactly
and the dice denominator is the constant 2.1 (identical to the reference's
own fp32 arithmetic to ~1e-7).  The device returns only the per-column sums
of mask*G[m,j]*r_m*r_j; host applies the affine map to the loss.

Device strategy (per core, 8 batches x 16 slots = 128 rows = 128 SBUF
partitions; every choice below is trace-measurement-driven, see v1-v3):
  - One full 128-row Gram via 512 accumulating PE matmuls (K=128 pixels per
    chunk).  The PE issue floor is ~34ns per LDWEIGHTS+MATMUL pair, so
    tile_position splits lose (v2 measured 2x worse); the rhs stream
    (1 col/cycle @2.4GHz warm) is the binding resource at ~53ns/chunk.
  - fp8e4m3 input (4x less HBM traffic; error cancels over 65536-element
    contractions).
  - Rows reordered so the 8 slot-0 rows come first: a column j is needed
    only for pairs m<j in the same batch, so slot-0 columns produce nothing
    -> rhs streams only columns 8..128 (120 data + ones), N=121 not 129.
  - Warmup: junk matmuls with no DMA dependency issue from program start,
    carrying the PE through the HAM activity window so real matmuls run at
    2.4GHz (v3 measured warm flip at 11.1us vs 14.4 without).
  - DMA: early tiles small (PE starts ~9.2us; early DMA only sustains
    ~0.3MB/us so large early tiles starve the PE - v3 measured 4us of
    boundary stalls), growing once the pipe is saturated.
  - Epilogue: r = 1/S (EPS is below fp32 ulp of S); the pair mask is folded
    into the broadcast matmul weights (maskT*r on DVE, then one bf16 matmul
    against a permuted identity); final partition-reduce via a ones-column
    matmul so the output is a single-partition [1,121] row -> one DMA
    descriptor (v3's [128,1] output = 128 4-byte descriptors cost ~6.5us in
    HBM write receipts).
Host: loss = (2*sum(out) + 0.1*npairs_total) / 2.1 / npairs_total.
"""

import os

import numpy as np

B, K, N = 64, 16, 65536
NCORES = 8
BPC = B // NCORES  # 8 batches per core
R = BPC * K  # 128 data rows per core
P = 128  # SBUF partitions
C_PER_P = N // P  # 512 pixel-chunks of 128
NC = R - BPC + 1  # 121 streamed columns: 120 slot>0 rows + ones
# column-halves of each supertile go to the two HWDGE queues concurrently
TILES = [32, 48, 64, 80, 96, 96, 96]  # sums to C_PER_P
WARMUP = int(os.environ.get("KERNEL_WARMUP", "34"))
SMOOTH = 0.1

_CACHE: dict = {}

# test.py reads this after calling kernel() to print HW exec time
LAST_RESULTS = None


def _row_order() -> np.ndarray:
    """Row permutation: the 8 slot-0 rows first, then slot 1..15 by batch."""
    first = [b * K for b in range(BPC)]
    rest = [b * K + k for b in range(BPC) for k in range(1, K)]
    return np.array(first + rest)


def _build_nc():
    import concourse.bacc as bacc
    import concourse.mybir as mybir
    import concourse.tile as tile

    f32 = mybir.dt.float32
    bf16 = mybir.dt.bfloat16
    xdt = mybir.dt.float8e4
    nc = bacc.Bacc("TRN2", target_bir_lowering=False)

    x = nc.dram_tensor("x", [P, C_PER_P, R + 1], xdt, kind="ExternalInput")
    consts = nc.dram_tensor("consts", [P, P + NC + 1], bf16, kind="ExternalInput")
    out_d = nc.dram_tensor("out", [1, NC], f32, kind="ExternalOutput")

    with tile.TileContext(nc) as tc:
        with (
            tc.tile_pool(name="xp", bufs=1) as xp,
            tc.tile_pool(name="sg", bufs=1) as sg,
            tc.tile_pool(name="ps", bufs=1, space="PSUM") as ps,
            tc.tile_pool(name="ps2", bufs=1, space="PSUM") as ps2,
            tc.tile_pool(name="wps", bufs=1, space="PSUM") as wps,
        ):
            # ---- PE warmup: junk matmuls with no DMA dependency ----
            wjunk = sg.tile([P, 16], xdt, name="wjunk")
            nc.gpsimd.memset(wjunk[:], 0.5)
            w_ps = wps.tile([P, 128], f32)
            for _ in range(WARMUP):
                nc.tensor.matmul(
                    w_ps[0:16, :],
                    wjunk[:, 0:16],
                    wjunk[:, 0:1].to_broadcast([P, 128]),
                    start=True,
                    stop=True,
                )

            # ---- input tiles (all resident; 66KB/partition at fp8) ----
            # each supertile's columns are split across the two HWDGE
            # queues (sync first half, scalar second half): two descriptor
            # generators run concurrently (one queue ramps at only
    # ~0.2MB/us on small descriptors) while arrival order stays equal to
            # the PE's consumption order (v5/v6 measured that whole-tile
            # alternation reorders arrivals and partition-splitting halves
            # throughput; both stall the PE into HAM re-throttle)
            xts = []
            off = 0
            for t, cc in enumerate(TILES):
                xt = xp.tile([P, cc, R + 1], xdt, name=f"xt{t}")
                half = cc // 2
                nc.sync.dma_start(
                    xt[:, 0:half, :], x[:, off : off + half, :]
                )
                nc.scalar.dma_start(
                    xt[:, half:cc, :], x[:, off + half : off + cc, :]
                )
                xts.append((xt, off, cc))
                off += cc
            # epilogue-only data, off the critical path
            consts_sb = sg.tile([P, P + NC + 1], bf16)
            nc.scalar.dma_start(consts_sb[:], consts[:, :])
            maskTc = consts_sb[:, 0:P]  # maskT[q, m] = mask[m, col(q)]
            identc = consts_sb[:, P : P + NC]  # ident[q, j] = (q == 8+j)
            onesc = consts_sb[:, P + NC : P + NC + 1]

            # ---- Gram accumulation: 512 x (lhsT [128,128], rhs [128,121]) --
            g_ps = ps.tile([P, NC], f32)
            for xt, off, cc in xts:
                for c in range(cc):
                    nc.tensor.matmul(
                        g_ps[:],
                        xt[:, c, 0:R],
                        xt[:, c, BPC : R + 1],
                        start=(off + c == 0),
                        stop=(off + c == C_PER_P - 1),
                    )

            # ---- epilogue ----
            # bf16 is plenty: the gram term is ~0.1% of the loss and the
            # tolerance is 2e-2; measured end-to-end error stays ~1e-6
            s_ps = g_ps[:, NC - 1 : NC]  # S[row] in PSUM (ones column)
            pack = sg.tile([P, 1], f32)
            nc.vector.reciprocal(pack[:], s_ps)  # r = 1/S
            rcol = pack[:, 0:1]
            with nc.allow_low_precision(reason="dice gram term, tol 2e-2"):
                # rBm[m, j] = mask[m, j] * r[col(j)] via one bf16 matmul
                # whose weights are maskT pre-scaled by r on DVE
                rmulb = sg.tile([P, P], bf16)
                nc.vector.tensor_scalar_mul(rmulb[:], maskTc, rcol)
                rBm = ps2.tile([P, NC], f32)
                nc.tensor.matmul(
                    rBm[:], rmulb[:], identc, start=True, stop=True
                )

                t1 = sg.tile([P, NC], bf16)
                nc.vector.tensor_scalar_mul(t1[:], g_ps[:], rcol)  # G*r_m
                nc.vector.tensor_mul(t1[:], t1[:], rBm[:])  # *r_j*mask

                # partition-reduce to one row: the output DMA is 1 descriptor
                nc.tensor.matmul(
                    w_ps[0:1, 0:NC], onesc, t1[:], start=True, stop=True
                )
            osb = sg.tile([P, NC], f32, name="osb")
            nc.vector.tensor_copy(out=osb[0:1, :], in_=w_ps[0:1, 0:NC])
            nc.sync.dma_start(out_d[:, :], osb[0:1, :])

    nc.compile()
    return nc


def _make_consts() -> np.ndarray:
    """[P, P+NC+1] bf16: maskT | permuted identity | ones column."""
    import ml_dtypes

    order = _row_order()
    m_row = order  # [128] original row id per out partition
    j_row = order[BPC:]  # [120] original row id per streamed data column
    mb, mk = m_row // K, m_row % K
    jb, jk = j_row // K, j_row % K
    mask = (mb[:, None] == jb[None, :]) & (mk[:, None] < jk[None, :])
    consts = np.zeros((P, P + NC + 1), dtype=ml_dtypes.bfloat16)
    # maskT[q, m] = mask[m, q-8] for q in 8..127
    consts[BPC:, 0:P] = mask.T.astype(ml_dtypes.bfloat16)
    for j in range(NC - 1):
        consts[BPC + j, P + j] = 1.0
    consts[:, P + NC] = 1.0
    return consts


def _shard_core(am_rows: np.ndarray) -> np.ndarray:
    """[8, 16, 65536] f32 -> [P, C_PER_P, R+1] fp8 device layout."""
    import ml_dtypes

    ndt = ml_dtypes.float8_e4m3
    xr = np.empty((R + 1, N), dtype=ndt)
    xr[0:R] = am_rows.reshape(R, N)[_row_order()].astype(ndt)
    xr[R] = 1.0
    # pixel n = p*C_PER_P + c ; [bk, p, c] -> [p, c, bk]
    xt = xr.reshape(R + 1, P, C_PER_P).transpose(1, 2, 0)
    return np.ascontiguousarray(xt)


def kernel(am: np.ndarray) -> np.ndarray:
    global LAST_RESULTS
    from concourse.bass_utils import run_bass_kernel_spmd

    if "nc" not in _CACHE:
        _CACHE["nc"] = _build_nc()
        _CACHE["consts"] = _make_consts()
    nc = _CACHE["nc"]
    consts = _CACHE["consts"]

    am = np.ascontiguousarray(np.asarray(am), dtype=np.float32)
    assert am.shape == (B, K, N)

    in_maps = []
    for core in range(NCORES):
        rows = am[core * BPC : (core + 1) * BPC]
        in_maps.append({"x": _shard_core(rows), "consts": consts})

    trace = bool(int(os.environ.get("KERNEL_TRACE", "0")))
    res = run_bass_kernel_spmd(
        nc, in_maps, core_ids=list(range(NCORES)), trace=trace
    )
    LAST_RESULTS = res

    tsum = float(
        np.sum(
            np.array([r["out"][0, :] for r in res.results], dtype=np.float64)
        )
    )
    npairs = K * (K - 1) // 2
    ntot = B * npairs  # 7680 masked pairs overall
    loss = (2.0 * tsum + SMOOTH * ntot) / (2.0 + SMOOTH) / ntot
    return np.float32(loss)
